# revision 26
# baseline (speedup 1.0000x reference)
"""YOLOv3 detection-decode kernel for 8 Trainium2 NeuronCores.

Data-parallel over batch (16 images -> 2 per core). Per (scale, anchor) the
kernel processes BOTH local images in one macro-iteration:
  1. One SWDGE DMA cast-loads the [85, HW] head slice for both images
     (f32->bf16) into rows 0:85 of a per-scale double-buffered tile
     res [88, 2*HW+pad]; rows 85:88 hold resident [ones; gx; gy] constants
     written once at startup.
  2. Per image, one in-place scalar tanh(x/2) pass over all 85 rows
     (sigmoid(x) = 0.5*tanh(x/2) + 0.5; the ACT engine is column-serial
     and ops wider than ~8k columns run at half rate, so split per image).
  3. Per image, ceil(HW/128) uniform 128-position chunks, one matmul each
     against a constant [88, 85] bf16 weight: transposes to [pos, 85],
     applies the 0.5/0.5 sigmoid affine + stride scaling, adds stride*grid
     offsets (ones/gx/gy rows), and passes t = tanh(x/2) through to the
     w/h columns 2/3. Chunk k covers positions {k + nchunk*i}; the last
     chunks of image 1 read junk from the pad columns, landing in PSUM
     partitions that are never stored.
  4. PSUM -> SBUF copies in 6-chunk batches cast to bf16; three cheap
     vector ops per image reconstruct w/h in place via
     anchor*e^x = 2*anchor/(1-t) - anchor (no scalar-engine exp, so the
     scalar stream never serializes consecutive iterations). Bulk HWDGE
     store of partitions [0:HW//nchunk] (nchunk*170B contiguous per
     partition) plus a single-descriptor SWDGE tail. bf16 output halves
     store bytes; host upcasts to f32.
"""

import math
import os
import sys

import numpy as np

sys.path.insert(0, "/opt/trn_rl_repo")

N_CORES = 8
B_TOTAL = 16
B_LOC = B_TOTAL // N_CORES  # 2

INP_DIM = 608
NC_CLS = 80  # num classes
CH = 85  # 5 + classes
K_ROWS = 88  # 85 data + ones + gx + gy

# (H, W, anchors[(w,h)x3]) per scale; strides 8/16/32
_SCALE_DEFS = [
    (76, 76, [(10.0, 13.0), (16.0, 30.0), (33.0, 23.0)]),
    (38, 38, [(30.0, 61.0), (62.0, 45.0), (59.0, 119.0)]),
    (19, 19, [(116.0, 90.0), (156.0, 198.0), (373.0, 326.0)]),
]


def _scales():
    out = []
    off = 0
    for h, w, anchors in _SCALE_DEFS:
        hw = h * w
        stride = INP_DIM // h
        nchunk = math.ceil(hw / 128)
        out.append(
            dict(
                H=h,
                W=w,
                HW=hw,
                stride=float(stride),
                anchors=anchors,
                off=off,
                nchunk=nchunk,
                padc=128 * nchunk - hw,
                pfull=hw // nchunk,
            )
        )
        off += 3 * hw
    return out, off


SCALES, N_ROWS = _scales()  # N_ROWS == 22743

# Smallest scale first (stores start flowing within ~2us) and smallest last
# (minimal store tail after the final load).
ITER_ORDER = [
    (2, 0),
    (0, 0), (0, 1), (0, 2),
    (1, 0), (1, 1), (1, 2),
    (2, 1), (2, 2),
]


def _make_weight(stride: float) -> np.ndarray:
    """[88, 85] matmul weight: transpose + sigmoid affine + grid/stride.
    All entries (0.5, 0.5*stride, stride, 1) are exact in bf16."""
    W = np.zeros((K_ROWS, CH), dtype=np.float32)
    W[0, 0] = W[1, 1] = 0.5 * stride  # x, y
    W[2, 2] = W[3, 3] = 1.0  # w, h: pass t = tanh(x/2) through
    for p in range(4, 85):
        W[p, p] = 0.5
    # ones row: sigmoid's +0.5 (stride-scaled for x/y; none for w/h)
    W[85, 0] = W[85, 1] = 0.5 * stride
    W[85, 4:] = 0.5
    W[86, 0] = stride  # gx row
    W[87, 1] = stride  # gy row
    return W


def _make_gridinit(h: int, w: int, padc: int) -> np.ndarray:
    """[3, B_LOC*HW + padc]: ones, grid_x, grid_y tiled per local image."""
    hw2 = B_LOC * h * w
    gi = np.zeros((3, hw2 + padc), dtype=np.float32)
    go = np.empty((3, h * w), dtype=np.float32)
    go[0] = 1.0
    go[1] = np.tile(np.arange(w, dtype=np.float32), h)
    go[2] = np.repeat(np.arange(h, dtype=np.float32), w)
    gi[:, 0:hw2] = np.tile(go, (1, B_LOC))
    return gi


def _patch_tile_drain():
    """The kernel-tail drain Tile emits carries one sem-wait per outstanding
    processor; this container's walrus rejects >1 sync wait on a Drain
    (CoreV3 setupSyncWait "Too many sync wait commands"). Split the waits
    across a chain of single-wait drains — same semantics, compiles."""
    import concourse.mybir as mybir
    from concourse import tile as _tile
    from concourse.vector_clock import ScopedClock

    if getattr(_tile.TileContext, "_drain_split_patched", False):
        return

    def _drain_and_barrier(self, tick_clock, wait_clock):
        drain_inst = self.nc.sync.drain()
        wait_clock.add_sem_waits(
            drain_inst.ins, ScopedClock({None: tick_clock.global_clock})
        )
        si = drain_inst.ins.sync_info
        if si is not None and len(si.on_wait) > 1:
            extra = list(si.on_wait[1:])
            del si.on_wait[1:]
            engines = [
                self.nc.sync,
                self.nc.scalar,
                self.nc.vector,
                self.nc.gpsimd,
                self.nc.tensor,
            ]
            for i, w in enumerate(extra):
                d2 = engines[i % len(engines)].drain()
                si2 = d2.ins.sync_info
                if si2 is None:
                    d2.ins.sync_info = mybir.SyncInfo(on_wait=[w], on_update=[])
                else:
                    si2.on_wait.append(w)
        self.nc.all_engine_barrier()
        assert self.sems is not None
        popped = self.nc._tile_sem_poison_stack.pop()
        assert popped is self._sem_poison
        self.nc._state.prepend_free_semaphores(
            [h.num for h in self.sems.allocated().values()]
        )

    _tile.TileContext._drain_and_barrier = _drain_and_barrier
    _tile.TileContext._drain_split_patched = True


_WAIT_CAP = 1


def _split_sync_waits(bir_json: bytes) -> bytes:
    """This container's walrus rejects instructions carrying more than one
    sync wait command. Move extra waits onto injected NoOps immediately
    before the instruction on the same engine queue (sequencers execute in
    order, so the combined wait semantics are identical)."""
    import json as _json

    d = _json.loads(bir_json)
    n = 0
    for f in d.get("functions", []):
        for bb in f.get("blocks", []):
            ins_list = bb.get("instructions", [])
            out = []
            for ins in ins_list:
                si = ins.get("sync_info")
                waits = (si or {}).get("on_wait") or []
                if len(waits) > _WAIT_CAP:
                    keep = waits[-_WAIT_CAP:]
                    extra = waits[: -_WAIT_CAP]
                    for i in range(0, len(extra), _WAIT_CAP):
                        n += 1
                        out.append(
                            {
                                "name": f"I-wsplit-{n}",
                                "opcode": "NoOp",
                                "engine": ins["engine"],
                                "ins": [],
                                "outs": [],
                                "bass_nofuse": True,
                                "sync_info": {
                                    "on_wait": extra[i : i + _WAIT_CAP],
                                    "on_update": [],
                                },
                            }
                        )
                    si["on_wait"] = keep
                out.append(ins)
            bb["instructions"] = out
    return _json.dumps(d).encode()


def _patch_compile():
    import concourse.bass_utils as bu

    if getattr(bu, "_wait_split_patched", False):
        return
    orig = bu.compile_bir_kernel

    def compile_bir_kernel_split(bir_json, tmpdir, neff_name="file.neff"):
        return orig(_split_sync_waits(bir_json), tmpdir, neff_name)

    bu.compile_bir_kernel = compile_bir_kernel_split
    bu._wait_split_patched = True
    import concourse.bass2jax as b2j

    b2j.compile_bir_kernel = compile_bir_kernel_split


def _build_program():
    import concourse.bass as bass
    import concourse.mybir as mybir
    from concourse.tile import TileContext

    _patch_tile_drain()
    _patch_compile()

    f32 = mybir.dt.float32
    bf16 = mybir.dt.bfloat16
    AF = mybir.ActivationFunctionType
    ALU = mybir.AluOpType

    nc = bass.Bass()

    x_dram = [
        nc.dram_tensor(f"x{s}", [B_LOC, 255, sc["HW"]], f32, kind="ExternalInput")
        for s, sc in enumerate(SCALES)
    ]
    w_dram = [
        nc.dram_tensor(f"w{s}", [K_ROWS, CH], bf16, kind="ExternalInput")
        for s in range(3)
    ]
    gi_dram = [
        nc.dram_tensor(
            f"gi{s}",
            [3, B_LOC * SCALES[s]["HW"] + SCALES[s]["padc"]],
            bf16,
            kind="ExternalInput",
        )
        for s in range(3)
    ]
    out = nc.dram_tensor("out", [B_LOC, N_ROWS, CH], bf16, kind="ExternalOutput")

    GROUP = 6  # transpose chunks per PSUM bank (6*85 = 510 <= 512 f32)

    with TileContext(nc) as tc:
        with (
            tc.tile_pool(name="consts", bufs=1) as cpool,
            tc.tile_pool(name="obuf", bufs=6) as opool,
            tc.tile_pool(name="psum", bufs=6, space="PSUM") as ppool,
        ):
            # Dependency-free dummy activation: forces the ACT table load at
            # stream start instead of behind the first iteration's load-wait.
            scratch = cpool.tile([1, 8], f32, tag="scratch")
            nc.scalar.activation(
                out=scratch[0:1, 0:1], in_=scratch[0:1, 0:1], func=AF.Tanh
            )

            # Allocate const tiles up front; DMA them lazily, interleaved
            # into the sync stream (iteration-0's input loads ride the
            # gpsimd stream, so the pipeline primes immediately either way).
            w_sb = [
                cpool.tile([K_ROWS, CH], bf16, tag=f"w{s}", name=f"w{s}")
                for s in range(3)
            ]
            res_t = [
                [
                    cpool.tile(
                        [K_ROWS, B_LOC * sc["HW"] + sc["padc"]],
                        bf16,
                        tag=f"res{s}_{bi}",
                        name=f"res{s}_{bi}",
                    )
                    for bi in range(2)
                ]
                for s, sc in enumerate(SCALES)
            ]
            for s in range(3):
                nc.sync.dma_start(out=w_sb[s][:], in_=w_dram[s][:])
                for bi in range(2):
                    nc.sync.dma_start(out=res_t[s][bi][85:88, :], in_=gi_dram[s][:])

            scale_ctr = [0, 0, 0]
            for s, a in ITER_ORDER:
                sc = SCALES[s]
                hw = sc["HW"]
                hw2 = B_LOC * hw
                nchunk = sc["nchunk"]
                pfull = sc["pfull"]
                tail = hw - pfull * nchunk
                bi = scale_ctr[s] % 2
                res = res_t[s][bi]
                scale_ctr[s] += 1
                c0 = 85 * a

                # Cast-loads (f32->bf16, SWDGE) per image straight into
                # the matmul tile. A dma_start's descriptors spread over
                # only largest-divisor-<=16(P) DMA queues, so split 80+5 to
                # hit all 16 queues instead of 85 -> 5. Loads ride the
                # gpsimd stream, stores the sync stream: a store's
                # compute-wait must never delay the next load's trigger.
                for b in range(B_LOC):
                    cb_ = b * hw
                    nc.gpsimd.dma_start(
                        out=res[0:80, cb_ : cb_ + hw],
                        in_=x_dram[s][b, c0 : c0 + 80, :],
                    )
                    nc.gpsimd.dma_start(
                        out=res[80:85, cb_ : cb_ + hw],
                        in_=x_dram[s][b, c0 + 80 : c0 + 85, :],
                    )

                base = sc["off"] + a * hw
                ncol = nchunk * CH
                # one obuf holds BOTH images' results so each store can be
                # a single two-region DMA (halves gpsimd trigger count)
                obuf = opool.tile([128, B_LOC * ncol], bf16, tag="obuf")
                for b in range(B_LOC):
                    cb = b * hw
                    ob0 = b * ncol
                    # sigmoid via tanh
                    nc.scalar.activation(
                        out=res[0:85, cb : cb + hw],
                        in_=res[0:85, cb : cb + hw],
                        func=AF.Tanh,
                        scale=0.5,
                    )
                    # Strided position chunks: chunk k covers positions
                    # {k + nchunk*i}, so PSUM/obuf partition i accumulates
                    # nchunk consecutive output rows -> the store DMA gets
                    # nchunk*170B contiguous per partition.
                    res_str = res[:, cb : cb + 128 * nchunk].rearrange(
                        "p (i r) -> p r i", r=nchunk
                    )
                    ngroups = math.ceil(nchunk / GROUP)
                    for g in range(ngroups):
                        k0 = g * GROUP
                        k1 = min(k0 + GROUP, nchunk)
                        psum = ppool.tile([128, GROUP * CH], f32, tag="ps")
                        for k in range(k0, k1):
                            nc.tensor.matmul(
                                psum[:, (k - k0) * CH : (k - k0) * CH + CH],
                                lhsT=res_str[:, k, :],
                                rhs=w_sb[s][:],
                                start=True,
                                stop=True,
                            )
                        wcols = (k1 - k0) * CH
                        nc.vector.tensor_copy(
                            out=obuf[:, ob0 + k0 * CH : ob0 + k0 * CH + wcols],
                            in_=psum[:, 0:wcols],
                        )

                    # w/h: anchor*e^x = 2*anchor/(1-t) - anchor, on the
                    # strided cols 2,3 (t = tanh(x/2) from the matmul).
                    # Junk partitions (never stored) may blow up harmlessly.
                    ob3 = obuf[:, ob0 : ob0 + ncol].rearrange(
                        "p (k c) -> p k c", c=CH
                    )
                    whv = ob3[:, 0:nchunk, 2:4]
                    with nc.allow_low_precision("bf16 w/h reconstruction"):
                        nc.vector.tensor_scalar(
                            out=whv,
                            in0=whv,
                            scalar1=-1.0,
                            scalar2=1.0,
                            op0=ALU.mult,
                            op1=ALU.add,
                        )
                        nc.vector.reciprocal(out=whv, in_=whv)
                        for col in (2, 3):
                            av = sc["anchors"][a][col - 2]
                            cv = ob3[:, 0:nchunk, col : col + 1]
                            nc.vector.tensor_scalar(
                                out=cv,
                                in0=cv,
                                scalar1=2.0 * av,
                                scalar2=-av,
                                op0=ALU.mult,
                                op1=ALU.add,
                            )

                # partition p <-> rows [base + p*nchunk, +nchunk): one
                # contiguous nchunk*170B descriptor per partition and image.
                # Split the bulk store at 112 partitions (112 = 16*7 -> all
                # 16 queues; pfull=125/120 would only spread over 5/15).
                psplit = 112 if s == 0 else pfull
                dst = out[:, base : base + psplit * nchunk, :].rearrange(
                    "b (p r) c -> p b (r c)", p=psplit
                )
                nc.sync.dma_start(
                    out=dst,
                    in_=obuf[0:psplit, :].rearrange("p (b c) -> p b c", c=ncol),
                )
                if psplit < pfull:
                    p2 = pfull - psplit
                    dst2 = out[
                        :, base + psplit * nchunk : base + pfull * nchunk, :
                    ].rearrange("b (p r) c -> p b (r c)", p=p2)
                    nc.sync.dma_start(
                        out=dst2,
                        in_=obuf[psplit:pfull, :].rearrange("p (b c) -> p b c", c=ncol),
                    )
                if tail:
                    nc.sync.dma_start(
                        out=out[:, base + pfull * nchunk : base + hw, :],
                        in_=obuf[pfull : pfull + 1, :].rearrange(
                            "p (b k c) -> p b k c", c=CH, k=nchunk
                        )[:, :, 0:tail, :],
                    )
    return nc


_PROGRAM = None
LAST_RESULT = None


def _get_program():
    global _PROGRAM
    if _PROGRAM is None:
        _PROGRAM = _build_program()
    return _PROGRAM


def kernel(x1: np.ndarray, x2: np.ndarray, x3: np.ndarray) -> np.ndarray:
    global LAST_RESULT
    from concourse.bass_utils import run_bass_kernel_spmd

    nc = _get_program()

    import ml_dtypes

    bf16 = ml_dtypes.bfloat16
    xs = [
        np.ascontiguousarray(x, dtype=np.float32).reshape(B_TOTAL, 255, sc["HW"])
        for x, sc in zip((x1, x2, x3), SCALES)
    ]
    w_consts = [_make_weight(sc["stride"]).astype(bf16) for sc in SCALES]
    gi_consts = [
        _make_gridinit(sc["H"], sc["W"], sc["padc"]).astype(bf16) for sc in SCALES
    ]

    in_maps = []
    for i in range(N_CORES):
        m = {}
        for s in range(3):
            m[f"x{s}"] = xs[s][i * B_LOC : (i + 1) * B_LOC]
            m[f"w{s}"] = w_consts[s]
            m[f"gi{s}"] = gi_consts[s]
        in_maps.append(m)

    LAST_RESULT = run_bass_kernel_spmd(nc, in_maps, core_ids=list(range(N_CORES)))
    return np.concatenate([r["out"] for r in LAST_RESULT.results], axis=0).astype(
        np.float32
    )
